# revision 2
# baseline (speedup 1.0000x reference)
"""K2Layer Trainium2 kernel: RMSNorm -> gated causal conv + low-rank decayed
linear attention -> proj -> residual -> RMSNorm -> MLP -> residual.

Sharding: pure data-parallel over batch (B=8 -> 1 batch element per core),
all parameters replicated. Attention uses a chunked linear-attention
decomposition (chunk C=128) so no S x S matrix is ever materialized:

  out[i] = sum_r q_a[i,r] * sum_{j<=i} gamma_r^(i-j) k[j,r] * v[j]

Per chunk: intra-chunk via A_cT[j,p] = sum_r (k gamma^(64-j))(q_a gamma^(p-64))
masked to p>=j, inter-chunk via a running [R,D] state. The depthwise causal
conv is folded into the same PSUM accumulation as a Toeplitz matmul.
RMSNorm weight vectors are folded into the adjacent weight matrices on host;
l2-normalization of q/k makes the rmsnorm row-scale drop out of the q/k path.

Self-contained: shapes hardcoded for B=8, S=1024, D=1024, R=16, K=4.
"""
import copy
import numpy as np

import concourse.bass as bass
import concourse.mybir as mybir
import concourse.tile as tile
from concourse.bass_utils import run_bass_kernel_spmd
from concourse.masks import make_identity

f32 = mybir.dt.float32
fr = mybir.dt.float32r
AF = mybir.ActivationFunctionType
ALU = mybir.AluOpType

B, S, D, R, KK = 8, 1024, 1024, 16, 4
R2 = 32 + R        # padded q|k row count: q at 0:R, k at 32:32+R
F = 4 * D
C = 128            # attention chunk length == tile height
NT = S // 128      # t tiles
ND = D // 128      # d tiles
NF = F // 128      # f tiles
NCH = S // C       # chunks
EPS_RMS = 1e-6
EPS_L2 = 1e-8
GAMMA_MIN, GAMMA_MAX = 0.15, 1.0
ALPHA_CAP = 1.0

_cache = {}
DEBUG = False


def _sigmoid(x):
    return 1.0 / (1.0 + np.exp(-x))


def _host_prep(inputs):
    u = np.asarray(inputs['u'], np.float64)
    v = np.asarray(inputs['v'], np.float64)
    norm1_w = np.asarray(inputs['norm1_w'], np.float64)
    norm2_w = np.asarray(inputs['norm2_w'], np.float64)
    proj_w = np.asarray(inputs['proj_w'], np.float64)
    mlp_w1 = np.asarray(inputs['mlp_w1'], np.float64)
    ker = np.asarray(inputs['k_base_kernel'], np.float64)

    gate = _sigmoid(float(inputs['k_base_gate_logit']))
    alpha = ALPHA_CAP * _sigmoid(np.asarray(inputs['alpha_logit'], np.float64))
    gamma = np.clip(_sigmoid(np.asarray(inputs['decay_logit'], np.float64)),
                    GAMMA_MIN, GAMMA_MAX)
    assert gamma.min() >= 0.25, "chunked gamma tables overflow fp32 below 0.25"

    p = np.arange(C)
    Gq = alpha[:, None] * gamma[:, None] ** (p[None, :] - 64)
    Gk = gamma[:, None] ** (64 - p[None, :])
    Gbar = gamma[:, None] ** (C + 64 - p[None, :])
    Gc = gamma[:, None] ** C

    Bc = np.zeros((C, C))
    for m in range(KK):
        idx = np.arange(C - m)
        Bc[idx, idx + m] = gate * ker[m]
    Bp = np.zeros((3, C))
    for q in range(3):
        for pp in range(3):
            m = pp - q + 3
            if 1 <= m <= 3:
                Bp[q, pp] = gate * ker[m]

    # q occupies partitions 0:R, k partitions 32:32+R (engine partition
    # accesses must start at 32-aligned bases; 16 is illegal)
    eones = np.zeros((R2, 2), np.float64)
    eones[:R, 0] = 1.0
    eones[32:32 + R, 1] = 1.0

    c = lambda x: np.ascontiguousarray(np.asarray(x, np.float32))
    w1_eff = mlp_w1 * norm2_w[:, None]                       # [D,F]
    # pack w1 so each [128,128] (dtile,ftile) tile is contiguous for DMA
    w1p = np.ascontiguousarray(
        w1_eff.reshape(ND, 128, NF, 128).transpose(2, 1, 0, 3)
        .reshape(NF, 128, ND * 128).astype(np.float32))
    return dict(
        uv_eff=c(np.concatenate(
            [u, np.zeros((D, 32 - R)), v], 1) * norm1_w[:, None]),
        proj_eff=c(proj_w.T * norm1_w[:, None]),
        w1p=w1p,
        w2=c(inputs['mlp_w2']),
        proj_b=c(np.reshape(inputs['proj_b'], (1, D))),
        b1=c(inputs['mlp_b1']),
        b2=c(np.reshape(inputs['mlp_b2'], (1, D))),
        Gq=c(Gq), Gk=c(Gk), Gbar=c(Gbar), Gc=c(Gc),
        Bc=c(Bc), Bp=c(Bp), eones=c(eones), e2=c(eones.T),
    )


def split_drain_waits(nc):
    """This walrus build allows at most ONE sem wait per instruction (any
    opcode). Peel excess waits onto preceding same-engine NoOp carriers."""
    n = 0
    for f in nc.m.functions:
        for bb in f.blocks:
            i = 0
            while i < len(bb.instructions):
                ins = bb.instructions[i]
                si = ins.sync_info
                if si and si.on_wait and len(si.on_wait) > 1:
                    waits = list(si.on_wait)
                    carriers = []
                    for k, w in enumerate(waits[:-1]):
                        nop = mybir.InstNoOp(name=f"{ins.name}-wpeel{k}", ins=[], outs=[])
                        nop.engine = ins.engine
                        si2 = copy.deepcopy(si)
                        si2.on_wait[:] = [w]
                        si2.on_update[:] = []
                        nop.sync_info = si2
                        carriers.append(nop)
                        n += 1
                    si.on_wait[:] = [waits[-1]]
                    bb.instructions[i:i] = carriers
                    i += len(carriers)
                i += 1
    return n


def _build_nc():
    nc = bass.Bass("TRN2")
    h_d = nc.dram_tensor("h", [S, D], f32, kind="ExternalInput")
    uv_d = nc.dram_tensor("uv_eff", [D, R2], fr, kind="ExternalInput")
    proj_d = nc.dram_tensor("proj_eff", [D, D], fr, kind="ExternalInput")
    w1_d = nc.dram_tensor("w1p", [NF, 128, ND * 128], fr, kind="ExternalInput")
    w2_d = nc.dram_tensor("w2", [F, D], fr, kind="ExternalInput")
    pb_d = nc.dram_tensor("proj_b", [1, D], fr, kind="ExternalInput")
    b1_d = nc.dram_tensor("b1", [F], f32, kind="ExternalInput")
    b2_d = nc.dram_tensor("b2", [1, D], fr, kind="ExternalInput")
    gq_d = nc.dram_tensor("Gq", [R, C], f32, kind="ExternalInput")
    gk_d = nc.dram_tensor("Gk", [R, C], f32, kind="ExternalInput")
    gbar_d = nc.dram_tensor("Gbar", [R, C], f32, kind="ExternalInput")
    gc_d = nc.dram_tensor("Gc", [R, 1], f32, kind="ExternalInput")
    bc_d = nc.dram_tensor("Bc", [C, C], f32, kind="ExternalInput")
    bp_d = nc.dram_tensor("Bp", [3, C], fr, kind="ExternalInput")
    eo_d = nc.dram_tensor("eones", [R2, 2], f32, kind="ExternalInput")
    e2_d = nc.dram_tensor("e2", [2, R2], f32, kind="ExternalInput")
    y_d = nc.dram_tensor("y", [S, D], f32, kind="ExternalOutput")
    dbg = {}
    if DEBUG:
        dbg['hs'] = nc.dram_tensor("dbg_hs", [S, D], fr, kind="ExternalOutput")
        dbg['qnT'] = nc.dram_tensor("dbg_qnT", [R2, S], f32, kind="ExternalOutput")
        dbg['omt'] = nc.dram_tensor("dbg_omt", [D, S], fr, kind="ExternalOutput")
        dbg['h2'] = nc.dram_tensor("dbg_h2", [S, D], f32, kind="ExternalOutput")
        dbg['hs2T'] = nc.dram_tensor("dbg_hs2T", [D, S], fr, kind="ExternalOutput")

    with tile.TileContext(nc) as tc:
        with (
            tc.tile_pool(name="const", bufs=1) as const,
            tc.tile_pool(name="qn", bufs=1) as qnp,
            tc.tile_pool(name="scal", bufs=2) as scal,
        ):
            # ---- constants ----
            ident = const.tile([128, 128], f32, tag="ident", name="ident")
            make_identity(nc, ident)
            trilm = const.tile([C, C], f32, tag="trilm", name="trilm")
            nc.gpsimd.memset(trilm, 0.0)
            nc.gpsimd.affine_select(
                out=trilm, in_=trilm, compare_op=ALU.is_gt, fill=1.0,
                base=0, pattern=[[-1, C]], channel_multiplier=1)
            gq_s = const.tile([R, C], f32, tag="gq", name="gq"); nc.sync.dma_start(out=gq_s, in_=gq_d[:, :])
            gk_s = const.tile([R, C], f32, tag="gk", name="gk"); nc.sync.dma_start(out=gk_s, in_=gk_d[:, :])
            gbar_s = const.tile([R, C], f32, tag="gbar", name="gbar"); nc.sync.dma_start(out=gbar_s, in_=gbar_d[:, :])
            gc_s = const.tile([R, 1], f32, tag="gc", name="gc"); nc.sync.dma_start(out=gc_s, in_=gc_d[:, :])
            bc_s = const.tile([C, C], f32, tag="bc", name="bc"); nc.sync.dma_start(out=bc_s, in_=bc_d[:, :])
            bp_s = const.tile([3, C], fr, tag="bp", name="bp"); nc.sync.dma_start(out=bp_s, in_=bp_d[:, :])
            eo_s = const.tile([R2, 2], f32, tag="eo", name="eo"); nc.sync.dma_start(out=eo_s, in_=eo_d[:, :])
            e2_s = const.tile([2, R2], f32, tag="e2", name="e2"); nc.sync.dma_start(out=e2_s, in_=e2_d[:, :])
            uv_s = const.tile([128, ND, R2], fr, tag="uv", name="uv")
            uv_r = uv_d.rearrange("(n p) r -> n p r", p=128)
            for k in range(ND):
                nc.sync.dma_start(out=uv_s[:, k, :], in_=uv_r[k])
            eps1 = const.tile([128, 1], f32, tag="eps1", name="eps1")
            nc.vector.memset(eps1, EPS_RMS)
            b1_s = const.tile([128, NF], f32, tag="b1", name="b1")
            b1_r = b1_d.rearrange("(n p) -> n p", p=128)
            for k in range(NF):
                nc.sync.dma_start(out=b1_s[:, k:k + 1], in_=b1_r[k][:, None])
            pb_row = const.tile([1, D], fr, tag="pbrow", name="pbrow")
            nc.sync.dma_start(out=pb_row, in_=pb_d[:, :])
            ones_f = const.tile([1, 128], f32, tag="onesf", name="onesf")
            nc.vector.memset(ones_f, 1.0)
            ones_r = const.tile([1, 128], fr, tag="onesr", name="onesr")
            nc.vector.tensor_copy(ones_r, ones_f)
            b2_row = const.tile([1, D], fr, tag="b2row", name="b2row")
            nc.sync.dma_start(out=b2_row, in_=b2_d[:, :])

            h_r = h_d.rearrange("(n p) d -> n p d", p=128)
            y_r = y_d.rearrange("(n p) d -> n p d", p=128)

            qnT = qnp.tile([R2, S], f32, tag="qnT", name="qnT")
            knT = qnp.tile([R, S], f32, tag="knT", name="knT")

            with tc.tile_pool(name="h2p", bufs=1) as h2p:
                with (
                    tc.tile_pool(name="hpb", bufs=1) as hpbp,
                    tc.tile_pool(name="omt", bufs=1) as omtp,
                ):
                    h2 = [h2p.tile([128, D], f32, tag=f"h2_{t}", name=f"h2_{t}") for t in range(NT)]
                    hpb = [hpbp.tile([128, D], f32, tag=f"h_{t}", name=f"h_{t}") for t in range(NT)]
                    omt = [omtp.tile([128, S], fr, tag=f"omt_{k}", name=f"omt_{k}") for k in range(ND)]

                    with tc.tile_pool(name="hs", bufs=1) as hsp:
                        hs = [hsp.tile([128, D], fr, tag=f"hs_{t}", name=f"hs_{t}") for t in range(NT)]

                        # ================= phase A: load h, rmsnorm scale, hs, hpb ====
                        for t in range(NT):
                            nc.sync.dma_start(out=hpb[t], in_=h_r[t])
                        for t in range(NT):
                            sq = scal.tile([128, D], f32, tag="sq", name="sq")
                            ssq = scal.tile([128, 1], f32, tag="ssq", name="ssq")
                            nc.gpsimd.tensor_tensor(out=sq, in0=hpb[t], in1=hpb[t],
                                                    op=ALU.mult)
                            nc.vector.tensor_reduce(ssq, sq, axis=mybir.AxisListType.X,
                                                    op=ALU.add)
                            sroot = scal.tile([128, 1], f32, tag="sroot", name="sroot")
                            nc.scalar.activation(sroot, ssq, AF.Sqrt,
                                                 bias=eps1, scale=1.0 / D)
                            srec = scal.tile([128, 1], f32, tag="srec", name="srec")
                            nc.vector.reciprocal(srec, sroot)
                            nc.vector.tensor_scalar_mul(hs[t], hpb[t], srec)

                        if DEBUG:
                            for t in range(NT):
                                nc.sync.dma_start(
                                    out=dbg['hs'].rearrange("(n p) d -> n p d", p=128)[t],
                                    in_=hs[t])
                        # ================= phase B: qk projection + l2 scales =========
                        with (
                            tc.tile_pool(name="hTt", bufs=3) as hTp,
                            tc.tile_pool(name="psB", bufs=1, space="PSUM") as psB,
                            tc.tile_pool(name="psB2", bufs=1, space="PSUM") as psB2,
                            tc.tile_pool(name="psTq", bufs=2, space="PSUM") as psTq,
                            tc.tile_pool(name="qtmp", bufs=1) as qtmp,
                        ):
                            qk_ps = psB.tile([R2, S], f32, tag="qk", name="qk")
                            for t in range(NT):
                                tsl = bass.ts(t, 128)
                                for k in range(ND):
                                    tp = psTq.tile([128, 128], f32, tag="tpq", name="tpq")
                                    nc.tensor.transpose(tp, hpb[t][:, bass.ts(k, 128)], ident)
                                    hsT_t = hTp.tile([128, 128], fr, tag="hsTt", name="hsTt")
                                    nc.scalar.copy(hsT_t, tp)
                                    nc.tensor.matmul(qk_ps[:, tsl], lhsT=uv_s[:, k, :],
                                                     rhs=hsT_t, start=(k == 0), stop=(k == ND - 1))
                            qkT = qtmp.tile([R2, S], f32, tag="qkT", name="qkT")
                            nc.vector.tensor_copy(qkT, qk_ps)
                            qk2 = qtmp.tile([R2, S], f32, tag="qk2", name="qk2")
                            nc.gpsimd.tensor_tensor(out=qk2, in0=qkT, in1=qkT,
                                                    op=ALU.mult)
                            ssq2_ps = psB2.tile([2, S], f32, tag="ssq2", name="ssq2")
                            for th in range(2):
                                nsl = bass.ts(th, 512)
                                nc.tensor.matmul(ssq2_ps[:, nsl], lhsT=eo_s, rhs=qk2[:, nsl],
                                                 start=True, stop=True)
                            srow = qtmp.tile([2, S], f32, tag="srow", name="srow")
                            nc.vector.tensor_scalar_max(srow, ssq2_ps, EPS_L2 * EPS_L2)
                            nc.vector.reciprocal(srow, srow)
                            nc.scalar.activation(srow, srow, AF.Sqrt)
                            sc_ps = psB2.tile([R2, S], f32, tag="sc32", name="sc32")
                            for th in range(2):
                                nsl = bass.ts(th, 512)
                                nc.tensor.matmul(sc_ps[:, nsl], lhsT=e2_s, rhs=srow[:, nsl],
                                                 start=True, stop=True)
                            nc.vector.tensor_tensor(out=qnT, in0=qkT, in1=sc_ps, op=ALU.mult)
                            # k rows to a base-0 tile: DVE tensor_tensor needs
                            # both SBUF inputs at the same base partition
                            nc.sync.dma_start(out=knT, in_=qnT[32:32 + R, :])
                            if DEBUG:
                                nc.sync.dma_start(out=dbg['qnT'][:, :], in_=qnT)

                        # ================= phase C: chunked attention + conv ==========
                        with (
                            tc.tile_pool(name="chk", bufs=3) as chk,
                            tc.tile_pool(name="chk1", bufs=2) as chk1,
                            tc.tile_pool(name="stp", bufs=2) as stp,
                            tc.tile_pool(name="psA", bufs=1, space="PSUM") as psA,
                            tc.tile_pool(name="psM", bufs=2, space="PSUM") as psM,
                            tc.tile_pool(name="psU", bufs=1, space="PSUM") as psU,
                            tc.tile_pool(name="psT", bufs=1, space="PSUM") as psT,
                        ):
                            st_prev = stp.tile([R, D], f32, tag="St", name="St")
                            nc.vector.memset(st_prev, 0.0)
                            for c in range(NCH):
                                csl = bass.ts(c, C)
                                qh = chk.tile([R, C], fr, tag="qh", name="qh")
                                kh = chk.tile([R, C], fr, tag="kh", name="kh")
                                kb = chk.tile([R, C], f32, tag="kb", name="kb")
                                nc.vector.tensor_tensor(out=qh, in0=qnT[:R, csl], in1=gq_s, op=ALU.mult)
                                nc.vector.tensor_tensor(out=kh, in0=knT[:, csl], in1=gk_s, op=ALU.mult)
                                nc.vector.tensor_tensor(out=kb, in0=knT[:, csl], in1=gbar_s, op=ALU.mult)
                                # A_cT then mask + conv toeplitz
                                a_ps = psA.tile([C, C], f32, tag="aps", name="aps")
                                nc.tensor.matmul(a_ps, lhsT=kh, rhs=qh, start=True, stop=True)
                                am = chk.tile([C, C], f32, tag="am", name="am")
                                nc.vector.tensor_tensor(out=am, in0=a_ps, in1=trilm,
                                                        op=ALU.mult)
                                am_r = chk.tile([C, C], fr, tag="amr", name="amr")
                                nc.vector.tensor_tensor(out=am_r, in0=am, in1=bc_s, op=ALU.add)
                                # KbarT via PE transpose
                                kbT_ps = psT.tile([C, R], f32, tag="kbT", name="kbT")
                                nc.tensor.transpose(kbT_ps, kb, ident[:R, :R])
                                kbT = chk.tile([C, R], fr, tag="kbTs", name="kbTs")
                                nc.vector.tensor_copy(kbT, kbT_ps)
                                # state contribution U_c
                                u_halves = []
                                for half in range(2):
                                    nsl = bass.ts(half, 512)
                                    u_ps = psU.tile([R, 512], f32, tag="ups", name="ups")
                                    nc.tensor.matmul(u_ps, lhsT=kbT, rhs=hs[c][:, nsl],
                                                     start=True, stop=True)
                                    u_halves.append(u_ps)
                                # last 3 rows of previous chunk's values at partition 0
                                # (matmul lhsT cannot sit at base_partition 125)
                                if c > 0:
                                    hs_tail = chk1.tile([3, D], fr, tag="hstail", name="hstail")
                                    nc.sync.dma_start(out=hs_tail, in_=hs[c - 1][125:128, :])
                                st_r = chk1.tile([R, D], fr, tag="str", name="str")
                                if c > 0:
                                    nc.vector.tensor_copy(st_r, st_prev)
                                # mixer in natural layout [p, d]; shared lhsT
                                # per chunk, then PE-transpose into out_midT
                                m_nat = []
                                for half in range(2):
                                    nsl = bass.ts(half, 512)
                                    m_ps = psM.tile([128, 512], f32, tag=f"mps{half}",
                                                    name=f"mps{half}")
                                    nc.tensor.matmul(m_ps, lhsT=am_r, rhs=hs[c][:, nsl],
                                                     start=True, stop=(c == 0))
                                    if c > 0:
                                        nc.tensor.matmul(m_ps, lhsT=qh, rhs=st_r[:, nsl],
                                                         start=False, stop=False)
                                        nc.tensor.matmul(m_ps, lhsT=bp_s,
                                                         rhs=hs_tail[:, nsl],
                                                         start=False, stop=True)
                                    mn = chk.tile([128, 512], f32, tag=f"mnat{half}",
                                                  name=f"mnat{half}")
                                    nc.vector.tensor_copy(mn, m_ps)
                                    m_nat.append(mn)
                                for k in range(ND):
                                    t_ps = psT.tile([128, 128], f32, tag="omtt", name="omtt")
                                    nc.tensor.transpose(
                                        t_ps, m_nat[k // 4][:, bass.ts(k % 4, 128)], ident)
                                    nc.scalar.copy(omt[k][:, csl], t_ps)
                                # state update
                                st_new = stp.tile([R, D], f32, tag="St", name="St")
                                nc.vector.tensor_scalar_mul(st_new, st_prev, gc_s)
                                for half in range(2):
                                    nsl = bass.ts(half, 512)
                                    nc.vector.tensor_tensor(out=st_new[:, nsl], in0=st_new[:, nsl],
                                                            in1=u_halves[half], op=ALU.add)
                                st_prev = st_new

                    if DEBUG:
                        for k in range(ND):
                            nc.sync.dma_start(
                                out=dbg['omt'].rearrange("(n p) d -> n p d", p=128)[k],
                                in_=omt[k])
                    # ================= phase D: proj + residual -> h2 =================
                    with (
                        tc.tile_pool(name="prj", bufs=1) as prjp,
                        tc.tile_pool(name="psP", bufs=2, space="PSUM") as psP,
                    ):
                        prj = [prjp.tile([128, D], fr, tag=f"prj_{k}", name=f"prj_{k}") for k in range(ND)]
                        prj_r = proj_d.rearrange("(n p) d -> n p d", p=128)
                        for k in range(ND):
                            nc.sync.dma_start(out=prj[k], in_=prj_r[k])
                        for t in range(NT):
                            tsl = bass.ts(t, 128)
                            p_ps0 = psP.tile([128, 512], f32, tag="pps0", name="pps0")
                            p_ps1 = psP.tile([128, 512], f32, tag="pps1", name="pps1")
                            for k in range(ND):
                                nc.tensor.matmul(p_ps0, lhsT=omt[k][:, tsl],
                                                 rhs=prj[k][:, 0:512],
                                                 start=(k == 0), stop=False)
                                nc.tensor.matmul(p_ps1, lhsT=omt[k][:, tsl],
                                                 rhs=prj[k][:, 512:1024],
                                                 start=(k == 0), stop=False)
                            nc.tensor.matmul(p_ps0, lhsT=ones_r, rhs=pb_row[:, 0:512],
                                             start=False, stop=True)
                            nc.tensor.matmul(p_ps1, lhsT=ones_r, rhs=pb_row[:, 512:1024],
                                             start=False, stop=True)
                            nc.vector.tensor_tensor(out=h2[t][:, 0:512], in0=p_ps0,
                                                    in1=hpb[t][:, 0:512], op=ALU.add)
                            nc.vector.tensor_tensor(out=h2[t][:, 512:1024], in0=p_ps1,
                                                    in1=hpb[t][:, 512:1024], op=ALU.add)

                if DEBUG:
                    for t in range(NT):
                        nc.sync.dma_start(
                            out=dbg['h2'].rearrange("(n p) d -> n p d", p=128)[t],
                            in_=h2[t])
                # ================= phase D2: rmsnorm2 + transpose =====================
                with tc.tile_pool(name="hs2T", bufs=1) as hs2Tp:
                    hs2T = [hs2Tp.tile([128, S], fr, tag=f"hs2T_{k}", name=f"hs2T_{k}") for k in range(ND)]
                    with (
                        tc.tile_pool(name="hs2", bufs=2) as hs2p,
                        tc.tile_pool(name="psT2", bufs=4, space="PSUM") as psT2,
                    ):
                        for t in range(NT):
                            sq = scal.tile([128, D], f32, tag="sq", name="sq")
                            ssq = scal.tile([128, 1], f32, tag="ssq", name="ssq")
                            nc.gpsimd.tensor_tensor(out=sq, in0=h2[t], in1=h2[t],
                                                    op=ALU.mult)
                            nc.vector.tensor_reduce(ssq, sq, axis=mybir.AxisListType.X,
                                                    op=ALU.add)
                            sroot = scal.tile([128, 1], f32, tag="sroot", name="sroot")
                            nc.scalar.activation(sroot, ssq, AF.Sqrt,
                                                 bias=eps1, scale=1.0 / D)
                            srec = scal.tile([128, 1], f32, tag="srec", name="srec")
                            nc.vector.reciprocal(srec, sroot)
                            hs2_t = hs2p.tile([128, D], f32, tag="hs2", name="hs2")
                            nc.vector.tensor_scalar_mul(hs2_t, h2[t], srec)
                            tsl = bass.ts(t, 128)
                            for k in range(ND):
                                tp_ps = psT2.tile([128, 128], f32, tag="tps", name="tps")
                                nc.tensor.transpose(tp_ps, hs2_t[:, bass.ts(k, 128)], ident)
                                nc.vector.tensor_copy(hs2T[k][:, tsl], tp_ps)

                    if DEBUG:
                        for k in range(ND):
                            nc.sync.dma_start(
                                out=dbg['hs2T'].rearrange("(n p) d -> n p d", p=128)[k],
                                in_=hs2T[k])
                    # ================= phase E: MLP (single pass, SBUF accum) ========
                    w2_r = w2_d.rearrange("(n p) d -> n p d", p=128)
                    with (
                        tc.tile_pool(name="gpool", bufs=3) as gp,
                        tc.tile_pool(name="w1s", bufs=3) as w1sp,
                        tc.tile_pool(name="w2s", bufs=3) as w2sp,
                        tc.tile_pool(name="psG", bufs=3, space="PSUM") as psG,
                        tc.tile_pool(name="psW", bufs=2, space="PSUM") as psW,
                    ):
                        for ft in range(NF):
                            w1_t = w1sp.tile([128, ND * 128], fr, tag="w1t", name="w1t")
                            nc.sync.dma_start(out=w1_t, in_=w1_d[ft])
                            g_ps = psG.tile([128, S], f32, tag="gps", name="gps")
                            for k in range(ND):
                                for nh in range(2):
                                    nc.tensor.matmul(
                                        g_ps[:, bass.ts(nh, 512)],
                                        lhsT=w1_t[:, bass.ts(k, 128)],
                                        rhs=hs2T[k][:, bass.ts(nh, 512)],
                                        start=(k == 0), stop=(k == ND - 1))
                            g_t = gp.tile([128, S], fr, tag="gt", name="gt")
                            nc.scalar.activation(g_t, g_ps, AF.Gelu_apprx_tanh,
                                                 bias=b1_s[:, ft:ft + 1])
                            w2_t = w2sp.tile([128, D], fr, tag="w2t", name="w2t")
                            nc.sync.dma_start(out=w2_t, in_=w2_r[ft])
                            for tq in range(NT):
                                for eh in range(2):
                                    p_w = psW.tile([128, 512], f32, tag="pw", name="pw")
                                    nc.tensor.matmul(
                                        p_w, lhsT=g_t[:, bass.ts(tq, 128)],
                                        rhs=w2_t[:, bass.ts(eh, 512)],
                                        start=True, stop=(ft > 0))
                                    if ft == 0:
                                        nc.tensor.matmul(
                                            p_w, lhsT=ones_r,
                                            rhs=b2_row[:, bass.ts(eh, 512)],
                                            start=False, stop=True)
                                    esl = bass.ts(eh, 512)
                                    nc.vector.tensor_tensor(out=h2[tq][:, esl],
                                                            in0=h2[tq][:, esl],
                                                            in1=p_w, op=ALU.add)
                        for t in range(NT):
                            nc.sync.dma_start(out=y_r[t], in_=h2[t])

    split_drain_waits(nc)
    return nc


def _make_in_maps(inputs):
    prep = _host_prep(inputs)
    h = np.ascontiguousarray(np.asarray(inputs['h'], np.float32))
    base = {k: prep[k] for k in prep}
    return [dict(base, h=np.ascontiguousarray(h[b])) for b in range(B)]


def kernel(**inputs):
    if 'nc' not in _cache:
        _cache['nc'] = _build_nc()
    nc = _cache['nc']
    in_maps = _make_in_maps(inputs)
    res = run_bass_kernel_spmd(nc, in_maps, core_ids=list(range(B)))
    return np.stack([res.results[b]['y'] for b in range(B)]).astype(np.float32)

